# revision 28
# baseline (speedup 1.0000x reference)
"""PlainGCN message passing on 8 TRN2 NeuronCores.

Computation (reference):
    deg = bincount(h); dis = deg**-0.5; norm = dis[t]*dis[h]
    out = relu(segment_sum(norm[:,None] * x[h], t, N))

Strategy (v13):
  - Fold dis[h] into x host-side: x2 = dis[:,None]*x (bf16). Then
    out[t] = relu(dis[t] * segment_sum(x2[h], t)) — the per-edge norm
    disappears; dis[t] is applied once per dest tile, fused with the
    ReLU on ScalarE (per-partition scale).
  - Shard edges by destination: core c owns dest nodes
    [c*N/8, (c+1)*N/8); x2 replicated in HBM.
  - Edge stream: per (block of TILE_BLOCK dest tiles, source bucket),
    the tiles' runs back to back with NO per-run padding (run length =
    max over cores of that run's edge count); only the span is padded
    to a multiple of 128 so gather chunks stay 128-aligned. Pad slots
    gather row 0 and carry tloc -1 (their one-hot rows are zero).
  - dma_gather x2 rows (256 B bf16) in <=1024-idx chunks,
    single_packet=True, round-robin over 4 SWDGE queues, deep SWDGE
    descriptor rings. The gather is Q7 descriptor-generation bound
    (~2.9 ns/idx serial on the Pool engine); all compute hides under
    it.
  - Segment-sum: one one-hot per (128-slot column, dest tile) pair —
    is_equal(iota, tloc) where tloc is -1 for slots outside the tile's
    run. A whole block's one-hots build in ONE DVE tensor_tensor with
    a stride-0-broadcast meta operand. TensorE accumulates full
    128-slot columns per tile into one PSUM bank (FWL bf16 weights),
    then Relu(dis_t * psum) on ScalarE, DMA out.
"""

import numpy as np

import concourse.bacc as bacc
import concourse.mybir as mybir
import concourse.tile as tile
from concourse.bass_utils import run_bass_kernel_spmd
from concourse.library_config import mlp as mlp_lib

P = 128
N_NODES = 100000
D_FEAT = 128
N_CORES = 8
BUCKET_W = 25000     # source-bucket width (< 32768 so idx fits int16)
TILE_BLOCK = 4       # dest tiles per block
GATHER_CHUNK = 1024  # max idxs per dma_gather (single_packet safe limit)


def _preprocess(x, h, t):
    n, d = x.shape
    assert (n, d) == (N_NODES, D_FEAT)
    npc = n // N_CORES
    n_tiles = -(-npc // P)
    n_buckets = -(-n // BUCKET_W)

    h = h.astype(np.int64)
    t = t.astype(np.int64)

    deg = np.bincount(h, minlength=n).astype(np.float64)
    dis = np.where(deg > 0, deg, 1.0) ** -0.5
    x2 = (x.astype(np.float64) * dis[:, None]).astype(np.float32)

    core = t // npc
    tloc = t - core * npc
    j = tloc // P
    tin = (tloc % P).astype(np.float32)
    b = h // BUCKET_W
    gidx_all = (h - b * BUCKET_W).astype(np.int16)

    counts = np.zeros((N_CORES, n_tiles, n_buckets), dtype=np.int64)
    np.add.at(counts, (core, j, b), 1)
    run_len = counts.max(axis=0)  # exact, no roundup

    n_blocks = -(-n_tiles // TILE_BLOCK)
    run_start = np.zeros((n_tiles, n_buckets), dtype=np.int64)
    gathers = []   # (bucket, start, length): <=GATHER_CHUNK, 128-aligned
    ohcols = []    # (col, tile): one one-hot per pair, block-contiguous
    tile_ohcols = [[] for _ in range(n_tiles)]  # per tile: (ohidx, col)
    blk_ohranges = []  # per block: (first ohidx, last ohidx+1)
    pos = 0
    for blk in range(n_blocks):
        tiles_blk = list(range(blk * TILE_BLOCK,
                               min((blk + 1) * TILE_BLOCK, n_tiles)))
        oh0 = len(ohcols)
        for bb in range(n_buckets):
            s0 = pos
            for jj in tiles_blk:
                run_start[jj, bb] = pos
                pos += int(run_len[jj, bb])
            # span layout pads to whole 128-columns (meta indexing), but
            # only the real content [s0, end16) is gathered — trailing pad
            # slots stay stale in SBUF and their one-hot rows are zero.
            end16 = s0 + (-(-(pos - s0) // 16) * 16)
            pos = -(-pos // P) * P
            c0 = s0
            while end16 - c0 > GATHER_CHUNK:
                gathers.append((bb, c0, GATHER_CHUNK))
                c0 += GATHER_CHUNK
            if end16 > c0:
                gathers.append((bb, c0, int(end16 - c0)))
            # one-hot columns: per tile, each column its run touches
            for jj in tiles_blk:
                s, r = int(run_start[jj, bb]), int(run_len[jj, bb])
                if r == 0:
                    continue
                for cc in range(s // P, (s + r - 1) // P + 1):
                    tile_ohcols[jj].append((len(ohcols), cc))
                    ohcols.append((cc, jj))
        blk_ohranges.append((oh0, len(ohcols)))
    e_pad = pos
    n_cols = e_pad // P
    n_oh = len(ohcols)
    max_blk_oh = max(b1 - b0 for (b0, b1) in blk_ohranges)

    # per-core streams
    order_key = (j // TILE_BLOCK) * (n_buckets * n_tiles) + b * n_tiles + j
    per_core = []
    for c in range(N_CORES):
        sel = np.nonzero(core == c)[0]
        sel = sel[np.argsort(order_key[sel], kind="stable")]
        jj = j[sel]
        bb2 = b[sel]
        key = jj * n_buckets + bb2
        change = np.r_[True, key[1:] != key[:-1]]
        grp_id = np.cumsum(change) - 1
        first_pos = np.nonzero(change)[0]
        within = np.arange(len(sel)) - first_pos[grp_id]
        posn = run_start[jj, bb2] + within

        gi = np.zeros(e_pad, dtype=np.int16)
        tf = np.full(e_pad, -1.0, dtype=np.float32)
        towner = np.full(e_pad, -1, dtype=np.int64)
        gi[posn] = gidx_all[sel]
        tf[posn] = tin[sel]
        towner[posn] = jj

        # wrap gather indices: [16, e/16] tiled x8 -> [128, e/16]
        wrap = np.tile(gi.reshape(e_pad // 16, 16).T, (8, 1)).astype(np.int16)

        # meta[p, k] for ohcol k=(col,tile): tloc of slot (col,p) if that
        # slot's edge targets `tile`, else -1
        tf2 = tf.reshape(n_cols, P)        # [C, 128]
        town2 = towner.reshape(n_cols, P)  # [C, 128]
        colarr = np.array([cc for (cc, _jj) in ohcols])
        jarr = np.array([_jj for (_cc, _jj) in ohcols])
        m3 = np.where(town2[colarr] == jarr[:, None],
                      tf2[colarr], -1.0)   # [n_oh, 128]
        meta = m3.T.astype(np.float32).copy()  # [128, n_oh]

        dnode = np.zeros(n_tiles * P, dtype=np.float32)
        dnode[:npc] = dis[c * npc:(c + 1) * npc].astype(np.float32)
        dis_t = dnode.reshape(n_tiles, P).T.copy()  # [128, n_tiles]

        per_core.append({"gidx": wrap, "meta": meta, "dis": dis_t})

    import ml_dtypes
    iota = np.tile(np.arange(P, dtype=np.float32), (P, 1))  # [128,128] f32
    x2b = x2.astype(ml_dtypes.bfloat16)

    sched = {
        "n": n, "d": d, "npc": npc, "n_tiles": n_tiles, "n_cols": n_cols,
        "e_pad": e_pad, "n_buckets": n_buckets, "n_blocks": n_blocks,
        "gathers": gathers, "ohcols": ohcols, "tile_ohcols": tile_ohcols,
        "blk_ohranges": blk_ohranges, "n_oh": n_oh,
        "max_blk_oh": max_blk_oh,
    }
    return sched, per_core, x2b, iota


def _build_program(sched, stage="full"):
    n, d, npc = sched["n"], sched["d"], sched["npc"]
    n_tiles, e_pad = sched["n_tiles"], sched["e_pad"]
    n_blocks, gathers = sched["n_blocks"], sched["gathers"]
    tile_ohcols = sched["tile_ohcols"]
    blk_ohranges = sched["blk_ohranges"]
    n_oh, max_blk_oh = sched["n_oh"], sched["max_blk_oh"]

    nc = bacc.Bacc("TRN2", target_bir_lowering=False, debug=False,
                   num_devices=N_CORES, num_swdge_queues=4,
                   dynamic_dma_scratch_size=65536)
    f32 = mybir.dt.float32
    bf16 = mybir.dt.bfloat16
    x_d = nc.dram_tensor("x2", [n, d], bf16, kind="ExternalInput")
    iota_d = nc.dram_tensor("iota", [P, P], f32, kind="ExternalInput")
    gidx_d = nc.dram_tensor("gidx", [P, e_pad // 16], mybir.dt.int16,
                            kind="ExternalInput")
    meta_d = nc.dram_tensor("meta", [P, n_oh], f32, kind="ExternalInput")
    dis_d = nc.dram_tensor("dis", [P, n_tiles], f32, kind="ExternalInput")
    y_d = nc.dram_tensor("y", [npc, d], f32, kind="ExternalOutput")

    nc.gpsimd.load_library(mlp_lib)

    gather_of_col = {}
    for gid, (bb, s0, ln) in enumerate(gathers):
        for cc in range(s0 // P, (s0 + ln + P - 1) // P):
            gather_of_col[cc] = (gid, s0 // P)

    relu = mybir.ActivationFunctionType.Relu

    with tile.TileContext(nc) as tc:
        with (
            tc.tile_pool(name="const", bufs=1) as cpool,
            tc.tile_pool(name="gather", bufs=28) as gpool,
            tc.tile_pool(name="onehot", bufs=4) as opool,
            tc.tile_pool(name="psum", bufs=7, space="PSUM") as ppool,
            tc.tile_pool(name="psiota", bufs=1, space="PSUM") as ipool,
            tc.tile_pool(name="outs", bufs=6) as ypool,
        ):
            gidx_t = cpool.tile([P, e_pad // 16], mybir.dt.int16, tag="gidx")
            nc.sync.dma_start(gidx_t[:], gidx_d[:, :])
            iota_t = cpool.tile([P, P], f32, tag="iota")
            nc.sync.dma_start(iota_t[:], iota_d[:, :])
            iota_p = ipool.tile([P, P], f32, tag="iop")
            # iota lives in PSUM: the one-hot build then reads only ONE
            # SBUF port (meta on rd0) — a two-SBUF-read tensor_tensor
            # borrows the POOL-shared port and stalls the Q7 descriptor
            # generators for the whole (multi-us) instruction.
            nc.vector.tensor_copy(iota_p[:], iota_t[:])
            meta_t = cpool.tile([P, n_oh], f32, tag="meta")
            nc.sync.dma_start(meta_t[:], meta_d[:, :])
            dis_t = cpool.tile([P, n_tiles], f32, tag="dis")
            nc.sync.dma_start(dis_t[:], dis_d[:, :])

            gtiles = {}

            def issue_gather(gid):
                bb, s0, ln = gathers[gid]
                base = bb * BUCKET_W
                rows = min(BUCKET_W, n - base)
                gt = gpool.tile([P, (GATHER_CHUNK // P) * d], bf16, tag="gt",
                                name=f"gt{gid}")
                if gid < 28:
                    # first use of each pool buffer: clear, so ungathered
                    # tail slots read 0.0 (not NaN bit patterns) under
                    # their zero one-hot rows; later reuses hold stale-
                    # but-finite x2 values.
                    nc.vector.memset(gt[:], 0.0)
                ncols_g = -(-ln // P)
                gt_3d = gt[:, :ncols_g * d].rearrange("p (c d) -> p c d",
                                                      d=d)
                nc.gpsimd.dma_gather(
                    gt_3d,
                    x_d[base:base + rows, :],
                    gidx_t[:, s0 // 16:(s0 + ln) // 16],
                    ln, ln, d,
                    single_packet=True,
                    queue_num=gid % 4,
                )
                gtiles[gid] = gt

            next_gather = 0
            for blk in range(n_blocks):
                tiles_blk = range(blk * TILE_BLOCK,
                                  min((blk + 1) * TILE_BLOCK, n_tiles))
                last_col = max(cc for jj in tiles_blk
                               for (_k, cc) in tile_ohcols[jj])
                while next_gather < len(gathers):
                    bb, s0, ln = gathers[next_gather]
                    if s0 // P > last_col:
                        break
                    issue_gather(next_gather)
                    next_gather += 1

                oh0, oh1 = blk_ohranges[blk]
                nbo = oh1 - oh0
                if stage != "gather":
                    # one DVE instruction builds the whole block's one-hots
                    ohblk = opool.tile([P, max_blk_oh * P], bf16,
                                       tag="ohb", name=f"ohb{blk}")
                    nc.vector.tensor_tensor(
                        ohblk[:, :nbo * P].rearrange(
                            "p (c f) -> p c f", f=P),
                        iota_p[:, None, :].broadcast_to([P, nbo, P]),
                        meta_t[:, oh0:oh1, None].broadcast_to([P, nbo, P]),
                        mybir.AluOpType.is_equal,
                    )

                for jj in tiles_blk:
                    ohlist = tile_ohcols[jj]
                    rows = min(P, npc - jj * P)
                    yt = ypool.tile([P, d], f32, tag="yt", name=f"yt{jj}")
                    if stage == "gather":
                        gid, _c0 = gather_of_col[ohlist[0][1]]
                        nc.vector.tensor_copy(yt[:], gtiles[gid][:, :d])
                        nc.sync.dma_start(y_d[jj * P:jj * P + rows, :],
                                          yt[:rows, :])
                        continue
                    pt = ppool.tile([P, d], f32, tag="ps", name=f"ps{jj}")
                    for si, (ohk, col) in enumerate(ohlist):
                        gid, col0 = gather_of_col[col]
                        gt = gtiles[gid]
                        col_l = col - col0
                        nc.tensor.matmul(
                            pt[:],
                            lhsT=ohblk[:, (ohk - oh0) * P:
                                       (ohk - oh0 + 1) * P],
                            rhs=gt[:, col_l * d:(col_l + 1) * d],
                            start=(si == 0),
                            stop=(si == len(ohlist) - 1),
                        )
                    nc.scalar.activation(yt[:], pt[:], relu,
                                         scale=dis_t[:, jj:jj + 1])
                    nc.sync.dma_start(y_d[jj * P:jj * P + rows, :],
                                      yt[:rows, :])

    nc.compile()
    return nc


def _run(x, h, t, trace=False, stage="full"):
    import time
    t0 = time.monotonic()
    sched, per_core, x2b, iota = _preprocess(np.asarray(x), np.asarray(h),
                                             np.asarray(t))
    t1 = time.monotonic()
    print(f"[kernel] preprocess {t1 - t0:.1f}s  e_pad={sched['e_pad']} "
          f"cols={sched['n_cols']} oh={sched['n_oh']} "
          f"gathers={len(sched['gathers'])}", flush=True)
    nc = _build_program(sched, stage=stage)
    t2 = time.monotonic()
    print(f"[kernel] build {t2 - t1:.1f}s", flush=True)
    in_maps = [
        {"x2": x2b, "iota": iota, "gidx": pc["gidx"], "meta": pc["meta"],
         "dis": pc["dis"]}
        for pc in per_core
    ]
    res = run_bass_kernel_spmd(nc, in_maps, core_ids=list(range(N_CORES)),
                               trace=trace)
    t3 = time.monotonic()
    print(f"[kernel] compile+run {t3 - t2:.1f}s", flush=True)
    y = np.concatenate([res.results[c]["y"] for c in range(N_CORES)], axis=0)
    return y, res


def kernel(x, h, t):
    y, _ = _run(np.asarray(x), np.asarray(h), np.asarray(t))
    return y


# revision 29
# speedup vs baseline: 1.0078x; 1.0078x over previous
"""PlainGCN message passing on 8 TRN2 NeuronCores.

Computation (reference):
    deg = bincount(h); dis = deg**-0.5; norm = dis[t]*dis[h]
    out = relu(segment_sum(norm[:,None] * x[h], t, N))

Strategy (v13):
  - Fold dis[h] into x host-side: x2 = dis[:,None]*x (bf16). Then
    out[t] = relu(dis[t] * segment_sum(x2[h], t)) — the per-edge norm
    disappears; dis[t] is applied once per dest tile, fused with the
    ReLU on ScalarE (per-partition scale).
  - Shard edges by destination: core c owns dest nodes
    [c*N/8, (c+1)*N/8); x2 replicated in HBM.
  - Edge stream: per (block of TILE_BLOCK dest tiles, source bucket),
    the tiles' runs back to back with NO per-run padding (run length =
    max over cores of that run's edge count); only the span is padded
    to a multiple of 128 so gather chunks stay 128-aligned. Pad slots
    gather row 0 and carry tloc -1 (their one-hot rows are zero).
  - dma_gather x2 rows (256 B bf16) in <=1024-idx chunks,
    single_packet=True, round-robin over 4 SWDGE queues, deep SWDGE
    descriptor rings. The gather is Q7 descriptor-generation bound
    (~2.9 ns/idx serial on the Pool engine); all compute hides under
    it.
  - Segment-sum: one one-hot per (128-slot column, dest tile) pair —
    is_equal(iota, tloc) where tloc is -1 for slots outside the tile's
    run. A whole block's one-hots build in ONE DVE tensor_tensor with
    a stride-0-broadcast meta operand. TensorE accumulates full
    128-slot columns per tile into one PSUM bank (FWL bf16 weights),
    then Relu(dis_t * psum) on ScalarE, DMA out.
"""

import numpy as np

import concourse.bacc as bacc
import concourse.mybir as mybir
import concourse.tile as tile
from concourse.bass_utils import run_bass_kernel_spmd
from concourse.library_config import mlp as mlp_lib

P = 128
N_NODES = 100000
D_FEAT = 128
N_CORES = 8
BUCKET_W = 25000     # source-bucket width (< 32768 so idx fits int16)
TILE_BLOCK = 4       # dest tiles per block
GATHER_CHUNK = 1024  # max idxs per dma_gather (single_packet safe limit)


def _preprocess(x, h, t):
    n, d = x.shape
    assert (n, d) == (N_NODES, D_FEAT)
    npc = n // N_CORES
    n_tiles = -(-npc // P)
    n_buckets = -(-n // BUCKET_W)

    h = h.astype(np.int64)
    t = t.astype(np.int64)

    deg = np.bincount(h, minlength=n).astype(np.float64)
    dis = np.where(deg > 0, deg, 1.0) ** -0.5
    x2 = (x.astype(np.float64) * dis[:, None]).astype(np.float32)

    core = t // npc
    b = h // BUCKET_W
    gidx_all = (h - b * BUCKET_W).astype(np.int16)

    # Per-core balanced node->tile assignment: flatten per-(tile,bucket)
    # edge counts so the SPMD schedule's max-over-cores padding shrinks.
    # newloc[node] = position of the node within its core's output block.
    tloc_orig = t - core * npc
    newloc_of = np.zeros(n, dtype=np.int64)
    for c in range(N_CORES):
        base = c * npc
        vdeg = np.zeros((npc, n_buckets), dtype=np.int64)
        selc = np.nonzero(core == c)[0]
        np.add.at(vdeg, (tloc_orig[selc], b[selc]), 1)
        order = np.argsort(-vdeg.sum(axis=1), kind="stable")
        loads = np.zeros((n_tiles, n_buckets), dtype=np.int64)
        slots = np.zeros(n_tiles, dtype=np.int64)
        fill = np.zeros(n_tiles, dtype=np.int64)
        cap = np.full(n_tiles, P, dtype=np.int64)
        cap[n_tiles - 1] = npc - (n_tiles - 1) * P
        newpos = np.zeros(npc, dtype=np.int64)
        for i in order:
            cand = slots < cap
            # tile minimizing the worst bucket load after adding node i
            score = (loads + vdeg[i][None, :]).max(axis=1)
            score[~cand] = 1 << 30
            jj_pick = int(np.argmin(score))
            loads[jj_pick] += vdeg[i]
            newpos[i] = jj_pick * P + fill[jj_pick]
            fill[jj_pick] += 1
            slots[jj_pick] += 1
        newloc_of[base:base + npc] = newpos

    tloc = newloc_of[t]
    j = tloc // P
    tin = (tloc % P).astype(np.float32)

    counts = np.zeros((N_CORES, n_tiles, n_buckets), dtype=np.int64)
    np.add.at(counts, (core, j, b), 1)
    run_len = counts.max(axis=0)  # exact, no roundup

    n_blocks = -(-n_tiles // TILE_BLOCK)
    run_start = np.zeros((n_tiles, n_buckets), dtype=np.int64)
    gathers = []   # (bucket, start, length): <=GATHER_CHUNK, 128-aligned
    ohcols = []    # (col, tile): one one-hot per pair, block-contiguous
    tile_ohcols = [[] for _ in range(n_tiles)]  # per tile: (ohidx, col)
    blk_ohranges = []  # per block: (first ohidx, last ohidx+1)
    pos = 0
    for blk in range(n_blocks):
        tiles_blk = list(range(blk * TILE_BLOCK,
                               min((blk + 1) * TILE_BLOCK, n_tiles)))
        oh0 = len(ohcols)
        for bb in range(n_buckets):
            s0 = pos
            for jj in tiles_blk:
                run_start[jj, bb] = pos
                pos += int(run_len[jj, bb])
            # span layout pads to whole 128-columns (meta indexing), but
            # only the real content [s0, end16) is gathered — trailing pad
            # slots stay stale in SBUF and their one-hot rows are zero.
            end16 = s0 + (-(-(pos - s0) // 16) * 16)
            pos = -(-pos // P) * P
            c0 = s0
            while end16 - c0 > GATHER_CHUNK:
                gathers.append((bb, c0, GATHER_CHUNK))
                c0 += GATHER_CHUNK
            if end16 > c0:
                gathers.append((bb, c0, int(end16 - c0)))
            # one-hot columns: per tile, each column its run touches
            for jj in tiles_blk:
                s, r = int(run_start[jj, bb]), int(run_len[jj, bb])
                if r == 0:
                    continue
                for cc in range(s // P, (s + r - 1) // P + 1):
                    tile_ohcols[jj].append((len(ohcols), cc))
                    ohcols.append((cc, jj))
        blk_ohranges.append((oh0, len(ohcols)))
    e_pad = pos
    n_cols = e_pad // P
    n_oh = len(ohcols)
    max_blk_oh = max(b1 - b0 for (b0, b1) in blk_ohranges)

    # per-core streams
    order_key = (j // TILE_BLOCK) * (n_buckets * n_tiles) + b * n_tiles + j
    per_core = []
    for c in range(N_CORES):
        sel = np.nonzero(core == c)[0]
        sel = sel[np.argsort(order_key[sel], kind="stable")]
        jj = j[sel]
        bb2 = b[sel]
        key = jj * n_buckets + bb2
        change = np.r_[True, key[1:] != key[:-1]]
        grp_id = np.cumsum(change) - 1
        first_pos = np.nonzero(change)[0]
        within = np.arange(len(sel)) - first_pos[grp_id]
        posn = run_start[jj, bb2] + within

        gi = np.zeros(e_pad, dtype=np.int16)
        tf = np.full(e_pad, -1.0, dtype=np.float32)
        towner = np.full(e_pad, -1, dtype=np.int64)
        gi[posn] = gidx_all[sel]
        tf[posn] = tin[sel]
        towner[posn] = jj

        # wrap gather indices: [16, e/16] tiled x8 -> [128, e/16]
        wrap = np.tile(gi.reshape(e_pad // 16, 16).T, (8, 1)).astype(np.int16)

        # meta[p, k] for ohcol k=(col,tile): tloc of slot (col,p) if that
        # slot's edge targets `tile`, else -1
        tf2 = tf.reshape(n_cols, P)        # [C, 128]
        town2 = towner.reshape(n_cols, P)  # [C, 128]
        colarr = np.array([cc for (cc, _jj) in ohcols])
        jarr = np.array([_jj for (_cc, _jj) in ohcols])
        m3 = np.where(town2[colarr] == jarr[:, None],
                      tf2[colarr], -1.0)   # [n_oh, 128]
        meta = m3.T.astype(np.float32).copy()  # [128, n_oh]

        dnode = np.zeros(n_tiles * P, dtype=np.float32)
        nl = newloc_of[c * npc:(c + 1) * npc]
        dnode[nl] = dis[c * npc:(c + 1) * npc].astype(np.float32)
        dis_t = dnode.reshape(n_tiles, P).T.copy()  # [128, n_tiles]

        per_core.append({"gidx": wrap, "meta": meta, "dis": dis_t})

    import ml_dtypes
    iota = np.tile(np.arange(P, dtype=np.float32), (P, 1))  # [128,128] f32
    x2b = x2.astype(ml_dtypes.bfloat16)

    sched = {
        "n": n, "d": d, "npc": npc, "n_tiles": n_tiles, "n_cols": n_cols,
        "e_pad": e_pad, "n_buckets": n_buckets, "n_blocks": n_blocks,
        "gathers": gathers, "ohcols": ohcols, "tile_ohcols": tile_ohcols,
        "blk_ohranges": blk_ohranges, "n_oh": n_oh,
        "max_blk_oh": max_blk_oh, "newloc_of": newloc_of,
    }
    return sched, per_core, x2b, iota


def _build_program(sched, stage="full"):
    n, d, npc = sched["n"], sched["d"], sched["npc"]
    n_tiles, e_pad = sched["n_tiles"], sched["e_pad"]
    n_blocks, gathers = sched["n_blocks"], sched["gathers"]
    tile_ohcols = sched["tile_ohcols"]
    blk_ohranges = sched["blk_ohranges"]
    n_oh, max_blk_oh = sched["n_oh"], sched["max_blk_oh"]

    nc = bacc.Bacc("TRN2", target_bir_lowering=False, debug=False,
                   num_devices=N_CORES, num_swdge_queues=4,
                   dynamic_dma_scratch_size=65536)
    f32 = mybir.dt.float32
    bf16 = mybir.dt.bfloat16
    x_d = nc.dram_tensor("x2", [n, d], bf16, kind="ExternalInput")
    iota_d = nc.dram_tensor("iota", [P, P], f32, kind="ExternalInput")
    gidx_d = nc.dram_tensor("gidx", [P, e_pad // 16], mybir.dt.int16,
                            kind="ExternalInput")
    meta_d = nc.dram_tensor("meta", [P, n_oh], f32, kind="ExternalInput")
    dis_d = nc.dram_tensor("dis", [P, n_tiles], f32, kind="ExternalInput")
    y_d = nc.dram_tensor("y", [npc, d], f32, kind="ExternalOutput")

    nc.gpsimd.load_library(mlp_lib)

    gather_of_col = {}
    for gid, (bb, s0, ln) in enumerate(gathers):
        for cc in range(s0 // P, (s0 + ln + P - 1) // P):
            gather_of_col[cc] = (gid, s0 // P)

    relu = mybir.ActivationFunctionType.Relu

    with tile.TileContext(nc) as tc:
        with (
            tc.tile_pool(name="const", bufs=1) as cpool,
            tc.tile_pool(name="gather", bufs=28) as gpool,
            tc.tile_pool(name="onehot", bufs=4) as opool,
            tc.tile_pool(name="psum", bufs=7, space="PSUM") as ppool,
            tc.tile_pool(name="psiota", bufs=1, space="PSUM") as ipool,
            tc.tile_pool(name="outs", bufs=6) as ypool,
        ):
            gidx_t = cpool.tile([P, e_pad // 16], mybir.dt.int16, tag="gidx")
            gsplit = (e_pad // 16) // 8
            nc.sync.dma_start(gidx_t[:, :gsplit], gidx_d[:, :gsplit])
            nc.sync.dma_start(gidx_t[:, gsplit:], gidx_d[:, gsplit:])
            iota_t = cpool.tile([P, P], f32, tag="iota")
            nc.sync.dma_start(iota_t[:], iota_d[:, :])
            iota_p = ipool.tile([P, P], f32, tag="iop")
            # iota lives in PSUM: the one-hot build then reads only ONE
            # SBUF port (meta on rd0) — a two-SBUF-read tensor_tensor
            # borrows the POOL-shared port and stalls the Q7 descriptor
            # generators for the whole (multi-us) instruction.
            nc.vector.tensor_copy(iota_p[:], iota_t[:])
            meta_t = cpool.tile([P, n_oh], f32, tag="meta")
            nc.sync.dma_start(meta_t[:], meta_d[:, :])
            dis_t = cpool.tile([P, n_tiles], f32, tag="dis")
            nc.sync.dma_start(dis_t[:], dis_d[:, :])

            gtiles = {}

            def issue_gather(gid):
                bb, s0, ln = gathers[gid]
                base = bb * BUCKET_W
                rows = min(BUCKET_W, n - base)
                gt = gpool.tile([P, (GATHER_CHUNK // P) * d], bf16, tag="gt",
                                name=f"gt{gid}")
                if gid < 28:
                    # first use of each pool buffer: clear, so ungathered
                    # tail slots read 0.0 (not NaN bit patterns) under
                    # their zero one-hot rows; later reuses hold stale-
                    # but-finite x2 values.
                    nc.vector.memset(gt[:], 0.0)
                ncols_g = -(-ln // P)
                gt_3d = gt[:, :ncols_g * d].rearrange("p (c d) -> p c d",
                                                      d=d)
                nc.gpsimd.dma_gather(
                    gt_3d,
                    x_d[base:base + rows, :],
                    gidx_t[:, s0 // 16:(s0 + ln) // 16],
                    ln, ln, d,
                    single_packet=True,
                    queue_num=gid % 4,
                )
                gtiles[gid] = gt

            next_gather = 0
            for blk in range(n_blocks):
                tiles_blk = range(blk * TILE_BLOCK,
                                  min((blk + 1) * TILE_BLOCK, n_tiles))
                last_col = max(cc for jj in tiles_blk
                               for (_k, cc) in tile_ohcols[jj])
                while next_gather < len(gathers):
                    bb, s0, ln = gathers[next_gather]
                    if s0 // P > last_col:
                        break
                    issue_gather(next_gather)
                    next_gather += 1

                oh0, oh1 = blk_ohranges[blk]
                nbo = oh1 - oh0
                if stage != "gather":
                    # one DVE instruction builds the whole block's one-hots
                    ohblk = opool.tile([P, max_blk_oh * P], bf16,
                                       tag="ohb", name=f"ohb{blk}")
                    nc.vector.tensor_tensor(
                        ohblk[:, :nbo * P].rearrange(
                            "p (c f) -> p c f", f=P),
                        iota_p[:, None, :].broadcast_to([P, nbo, P]),
                        meta_t[:, oh0:oh1, None].broadcast_to([P, nbo, P]),
                        mybir.AluOpType.is_equal,
                    )

                for jj in tiles_blk:
                    ohlist = tile_ohcols[jj]
                    rows = min(P, npc - jj * P)
                    yt = ypool.tile([P, d], f32, tag="yt", name=f"yt{jj}")
                    if stage == "gather":
                        gid, _c0 = gather_of_col[ohlist[0][1]]
                        nc.vector.tensor_copy(yt[:], gtiles[gid][:, :d])
                        nc.sync.dma_start(y_d[jj * P:jj * P + rows, :],
                                          yt[:rows, :])
                        continue
                    pt = ppool.tile([P, d], f32, tag="ps", name=f"ps{jj}")
                    for si, (ohk, col) in enumerate(ohlist):
                        gid, col0 = gather_of_col[col]
                        gt = gtiles[gid]
                        col_l = col - col0
                        nc.tensor.matmul(
                            pt[:],
                            lhsT=ohblk[:, (ohk - oh0) * P:
                                       (ohk - oh0 + 1) * P],
                            rhs=gt[:, col_l * d:(col_l + 1) * d],
                            start=(si == 0),
                            stop=(si == len(ohlist) - 1),
                        )
                    nc.scalar.activation(yt[:], pt[:], relu,
                                         scale=dis_t[:, jj:jj + 1])
                    nc.sync.dma_start(y_d[jj * P:jj * P + rows, :],
                                      yt[:rows, :])

    nc.compile()
    return nc


def _run(x, h, t, trace=False, stage="full"):
    import time
    t0 = time.monotonic()
    sched, per_core, x2b, iota = _preprocess(np.asarray(x), np.asarray(h),
                                             np.asarray(t))
    t1 = time.monotonic()
    print(f"[kernel] preprocess {t1 - t0:.1f}s  e_pad={sched['e_pad']} "
          f"cols={sched['n_cols']} oh={sched['n_oh']} "
          f"gathers={len(sched['gathers'])}", flush=True)
    nc = _build_program(sched, stage=stage)
    t2 = time.monotonic()
    print(f"[kernel] build {t2 - t1:.1f}s", flush=True)
    in_maps = [
        {"x2": x2b, "iota": iota, "gidx": pc["gidx"], "meta": pc["meta"],
         "dis": pc["dis"]}
        for pc in per_core
    ]
    res = run_bass_kernel_spmd(nc, in_maps, core_ids=list(range(N_CORES)),
                               trace=trace)
    t3 = time.monotonic()
    print(f"[kernel] compile+run {t3 - t2:.1f}s", flush=True)
    ycat = np.concatenate([res.results[c]["y"] for c in range(N_CORES)],
                          axis=0)
    # row (c*npc + newloc) holds node (c*npc + origloc): unpermute
    npc = sched["npc"]
    newloc = sched["newloc_of"]
    src = (np.arange(len(newloc)) // npc) * npc + newloc
    y = ycat[src]
    return y, res


def kernel(x, h, t):
    y, _ = _run(np.asarray(x), np.asarray(h), np.asarray(t))
    return y
